# revision 39
# baseline (speedup 1.0000x reference)
"""Trainium2 Bass kernel for nn_DLRMambaBackbone.

Strategy: data-parallel over batch (8 batch elems -> 8 NeuronCores). Each core
runs the full backbone for one batch element:
  stem conv(s2) + BN/SiLU -> 3x SS2D selective-scan blocks -> p4/p5 convs.

Per-core layout: activations channel-major [C, L] in SBUF. The selective scan
runs on the vector engine's tensor_tensor_scan over 16 [128, L] tiles whose
partitions are (r, n) lane pairs (2 r's x 64 n's per tile). dA = exp(dt * A)
is built by a K=6 bf16 hi/lo split matmul (exact to ~2^-17) + ACT exp.
The state sequence is DMA'd out rn-major as bf16 and transposed/upcast on the
host (a device-side transpose would need a slow scatter DMA or extra engine
passes).
"""

import os
import sys

import numpy as np

for _p in ("/opt/trn_rl_repo", "/root/.axon_site/_ro/trn_rl_repo"):
    if os.path.isdir(_p) and _p not in sys.path:
        sys.path.insert(0, _p)

import ml_dtypes

BF16 = ml_dtypes.bfloat16

# ---------------- problem geometry (hardcoded) ----------------
B = 8
CIN, HIN, WIN = 32, 112, 112
E = 64            # channels in SS2D trunk
H1 = W1 = 56
L = H1 * W1       # 3136 tokens
R, N, NB = 32, 64, 3
C4, C5 = 128, 256
H4 = W4 = 28
H5 = W5 = 14
NK = 16           # rn tiles: 16 x (2 r's x 64 n's)
CHUNK = 392       # 7 rows of 56 pixels; 3136 = 8*392
NCHUNK = 8
HALF = L // 2     # 1568
EPS = 1e-5


def _f32(x):
    return np.ascontiguousarray(np.asarray(x, dtype=np.float32))


def _bf16(x):
    return np.ascontiguousarray(np.asarray(x, dtype=np.float32).astype(BF16))


def _prep_consts(inp):
    """Host-side weight transforms (all tiny)."""
    c = {}
    # stem: fold BN scale into conv weights, BN shift into bias
    sg = inp["stem_g"] / np.sqrt(inp["stem_v"] + EPS)
    sb = inp["stem_b"] - inp["stem_m"] * sg
    w = inp["stem_w"]  # [E, CIN, 3, 3]
    stemw = np.empty((9, CIN, E), np.float32)
    for di in range(3):
        for dj in range(3):
            stemw[di * 3 + dj] = (w[:, :, di, dj] * sg[:, None]).T
    c["stemw"] = _f32(stemw)
    c["stemb"] = _f32(sb.reshape(E, 1))

    # SS2D blocks
    A = -np.exp(np.asarray(inp["blk_A_log"], np.float64))  # [NB, R, N]
    A = A.astype(np.float32)
    A_hi = A.astype(BF16).astype(np.float32)
    A_lo = (A - A_hi).astype(BF16).astype(np.float32)
    # K=96 masked lhsT per (block, k-tile): rows 0-31 pair with dthi (coef A_hi),
    # rows 32-63 with dtlo (A_hi), rows 64-95 with dthi again (A_lo).
    amask = np.zeros((NB, NK, 96, 128), np.float32)
    for i in range(NB):
        for k in range(NK):
            for j in range(2):
                r = 2 * k + j
                cs = slice(64 * j, 64 * j + 64)
                amask[i, k, r, cs] = A_hi[i, r]
                amask[i, k, 32 + r, cs] = A_hi[i, r]
                amask[i, k, 64 + r, cs] = A_lo[i, r]
    c["amask"] = _bf16(amask.transpose(2, 0, 1, 3))  # -> [96, NB, NK, 128]

    wud = np.empty((E, NB, 64), np.float32)   # cols: [u (R) | dtraw (R)]
    wbc = np.empty((E, NB, 128), np.float32)  # cols: [B (N) | C (N)]
    outwT = np.empty((R, NB, E), np.float32)
    for i in range(NB):
        wud[:, i, :R] = inp["blk_in_w"][i].T
        wud[:, i, R:] = inp["blk_dt_w"][i].T
        wbc[:, i, :N] = inp["blk_B_w"][i].T
        wbc[:, i, N:] = inp["blk_C_w"][i].T
        outwT[:, i, :] = inp["blk_out_w"][i].T
    c["wud"] = _f32(wud)
    c["wbc"] = _f32(wbc)
    c["outwT"] = _bf16(outwT)
    c["dtb"] = _f32(np.asarray(inp["blk_dt_b"]).T.reshape(R, NB))
    c["dvec"] = _f32(np.asarray(inp["blk_D"]).T.reshape(R, NB))

    # per-k selector: column 2k sums partitions 0-63, column 2k+1 sums 64-127
    sel32 = np.zeros((NK, 128, R), np.float32)
    for k in range(NK):
        sel32[k, :64, 2 * k] = 1.0
        sel32[k, 64:, 2 * k + 1] = 1.0
    c["sel32"] = _bf16(sel32.transpose(1, 0, 2))  # -> [128, NK, R]

    # p4 / p5 convs
    sg4 = inp["p4_g"] / np.sqrt(inp["p4_v"] + EPS)
    c["p4b"] = _f32((inp["p4_b"] - inp["p4_m"] * sg4).reshape(C4, 1))
    w4 = inp["p4_w"]  # [C4, E, 3, 3]
    p4w = np.empty((9, E, C4), np.float32)
    for di in range(3):
        for dj in range(3):
            p4w[di * 3 + dj] = (w4[:, :, di, dj] * sg4[:, None]).T
    c["p4w"] = _f32(p4w)

    sg5 = inp["p5_g"] / np.sqrt(inp["p5_v"] + EPS)
    b5 = inp["p5_b"] - inp["p5_m"] * sg5
    w5 = inp["p5_w"]  # [C5, C4, 3, 3]
    p5w = np.empty((2, 9, C4, 128), np.float32)
    for h in range(2):
        for di in range(3):
            for dj in range(3):
                p5w[h, di * 3 + dj] = (
                    w5[h * 128 : (h + 1) * 128, :, di, dj]
                    * sg5[h * 128 : (h + 1) * 128, None]
                ).T
    c["p5w"] = _f32(p5w)
    c["p5b"] = _f32(np.asarray(b5).reshape(2, 128, 1))
    return c


# ---------------- device program ----------------

def _pad_view(t, h):
    """View a [C, h*h] padded tile as tap-sliceable [C, hh, 2, hh, 2]-ish."""
    return t.rearrange("c (h w) -> c h w", h=h, w=h)


def _tap_view(tr, di, dj, hout):
    """Strided tap view [C, hout, hout] of padded image for a 3x3/stride2 conv."""
    hin = 2 * hout
    v = tr[:, di : di + hin, dj : dj + hin]
    v = v.rearrange("c (i a) (j b) -> c i a j b", a=2, b=2)
    return v[:, :, 0:1, :, 0:1]  # [C, hout, 1, hout, 1]


def build_program():
    import concourse.mybir as mybir
    import concourse.tile as tile
    from concourse import bacc

    f32 = mybir.dt.float32
    bf16 = mybir.dt.bfloat16
    AF = mybir.ActivationFunctionType
    OP = mybir.AluOpType

    nc = bacc.Bacc("TRN2", target_bir_lowering=False, debug=False)

    # --- DRAM tensors
    def din(name, shape, dt=f32):
        return nc.dram_tensor(name, list(shape), dt, kind="ExternalInput").ap()

    def dout(name, shape, dt=f32):
        return nc.dram_tensor(name, list(shape), dt, kind="ExternalOutput").ap()

    x_in = din("x_in", (CIN, HIN * WIN))
    stemw = din("stemw", (9, CIN, E))
    stemb = din("stemb", (E, 1))
    wud = din("wud", (E, NB, 64))
    wbc = din("wbc", (E, NB, 128))
    dtb = din("dtb", (R, NB))
    dvec = din("dvec", (R, NB))
    amask = din("amask", (96, NB, NK, 128), bf16)
    outwT = din("outwT", (R, NB, E), bf16)
    sel32 = din("sel32", (128, NK, R), bf16)
    p4w = din("p4w", (9, E, C4))
    p4b = din("p4b", (C4, 1))
    p5w = din("p5w", (2, 9, C4, 128))
    p5b = din("p5b", (2, C4, 1))

    dtu_d = nc.dram_tensor("dtu_d", [NB, R, L], bf16, kind="Internal").ap()
    states_o = dout("states_o", (NB, NK, 128, L), bf16)
    p3_o = dout("p3_o", (E, L))
    p4_o = dout("p4_o", (C4, H4 * W4))
    p5_o = dout("p5_o", (2, 128, H5 * W5))

    from contextlib import ExitStack

    with tile.TileContext(nc) as tc, ExitStack() as ctx:
        consts = ctx.enter_context(tc.tile_pool(name="consts", bufs=1))
        big = ctx.enter_context(tc.tile_pool(name="big", bufs=1))
        ps_a = ctx.enter_context(tc.tile_pool(name="ps_a", bufs=1, space="PSUM"))
        ps_b = ctx.enter_context(tc.tile_pool(name="ps_b", bufs=1, space="PSUM"))
        ps_m = ctx.enter_context(tc.tile_pool(name="ps_m", bufs=2, space="PSUM"))
        ps_y = ctx.enter_context(tc.tile_pool(name="ps_y", bufs=4, space="PSUM"))

        # --- load constants to SBUF
        def load_const(name, src, shape, dt, rearr=None):
            t = consts.tile(list(shape), dt, tag=name)
            s = src.rearrange(rearr) if rearr else src
            nc.sync.dma_start(out=t, in_=s)
            return t

        stemw_s = load_const("stemw", stemw, (CIN, 9, E), f32, "t c o -> c t o")
        stemb_s = load_const("stemb", stemb, (E, 1), f32)
        wud_s = load_const("wud", wud, (E, NB, 64), f32)
        wbc_s = load_const("wbc", wbc, (E, NB, 128), f32)
        dtb_s = load_const("dtb", dtb, (R, NB), f32)
        dvec_s = load_const("dvec", dvec, (R, NB), f32)
        amask_s = load_const("amask", amask, (96, NB, NK, 128), bf16)
        outwT_s = load_const("outwT", outwT, (R, NB, E), bf16)
        sel32_s = load_const("sel32", sel32, (128, NK, R), bf16)
        p4w_s = load_const("p4w", p4w, (E, 9, C4), f32, "t c o -> c t o")
        p4b_s = load_const("p4b", p4b, (C4, 1), f32)
        p5w_s = load_const("p5w", p5w, (C4, 2, 9, 128), f32, "h t c o -> c h t o")
        p5b_s = load_const("p5b", p5b, (C4, 2), f32, "h c one -> c (h one)")

        # --- stem conv
        X0 = big.tile([E, L], f32, tag="X0")
        X1 = big.tile([E, L], f32, tag="X1")
        Xs = [X0, X1]

        with tc.tile_pool(name="stempad", bufs=1) as stempad:
            xpad = stempad.tile([CIN, 114 * 114], f32, tag="xpad")
            nc.vector.memset(xpad, 0.0)
            xpad_r = _pad_view(xpad, 114)
            xtmp = stempad.tile([CIN, HIN * WIN], f32, tag="xtmp")
            nc.sync.dma_start(out=xtmp, in_=x_in)
            nc.vector.tensor_copy(
                xpad_r[:, 1:113, 1:113],
                xtmp.rearrange("c (h w) -> c h w", h=HIN, w=WIN),
            )
            for c8 in range(NCHUNK):
                ps = ps_a.tile([E, CHUNK], f32, tag="ps_a")
                for tap in range(9):
                    di, dj = tap // 3, tap % 3
                    v = _tap_view(xpad_r, di, dj, H1)  # [CIN, 56, 1, 56, 1]
                    rhs = v[:, 7 * c8 : 7 * c8 + 7]
                    nc.tensor.matmul(
                        ps, stemw_s[:, tap], rhs, start=(tap == 0), stop=(tap == 8)
                    )
                # SiLU = y * sigmoid(y), y = conv + bias
                yb = stempad.tile([E, CHUNK], f32, tag="yb")
                sg = stempad.tile([E, CHUNK], f32, tag="sg")
                nc.scalar.activation(out=yb, in_=ps, func=AF.Identity, bias=stemb_s)
                nc.scalar.activation(out=sg, in_=ps, func=AF.Sigmoid, bias=stemb_s)
                nc.vector.tensor_mul(
                    X0[:, CHUNK * c8 : CHUNK * (c8 + 1)], yb, sg
                )

        # --- SS2D blocks
        blk = ctx.enter_context(tc.tile_pool(name="blk", bufs=1))
        hpool = ctx.enter_context(tc.tile_pool(name="hpool", bufs=4))
        work = ctx.enter_context(tc.tile_pool(name="work", bufs=3))
        zpool = ctx.enter_context(tc.tile_pool(name="zp", bufs=3))
        for i in range(NB):
            Xin = Xs[i % 2]
            Xout = Xs[(i + 1) % 2]

            dt_f = blk.tile([R, L], f32, tag="dt_f")
            u_f = blk.tile([R, L], bf16, tag="u_f")
            bc_sb = blk.tile([128, L], bf16, tag="bc_sb")
            for c8 in range(NCHUNK):
                sl = slice(CHUNK * c8, CHUNK * (c8 + 1))
                ps1 = ps_a.tile([64, CHUNK], f32, tag="ps_a")
                nc.tensor.matmul(ps1, wud_s[:, i], Xin[:, sl], start=True, stop=True)
                ps2 = ps_b.tile([128, CHUNK], f32, tag="ps_b")
                nc.tensor.matmul(ps2, wbc_s[:, i], Xin[:, sl], start=True, stop=True)
                # softplus = ln(1 + exp(raw + bias)); stays in the ln/exp
                # ACT table sets (avoids softplus-set thrash, sim-supported)
                et = zpool.tile([R, CHUNK], f32, tag="et")
                nc.scalar.activation(
                    out=et, in_=ps1[R:64], func=AF.Exp,
                    bias=dtb_s[:, i : i + 1], scale=1.0,
                )
                nc.scalar.activation(
                    out=dt_f[:, sl], in_=et, func=AF.Ln, bias=1.0, scale=1.0,
                )
                nc.scalar.activation(out=u_f[:, sl], in_=ps1[0:R], func=AF.Copy)
                nc.scalar.activation(out=bc_sb[:, sl], in_=ps2, func=AF.Copy)

            # dtcat [96, L]: rows 0-31 dt-hi, 32-63 dt-lo, 64-95 dt-hi (again);
            # pairs with the K=96 masked lhsT amask_s for an exact-ish dt*A.
            dtcat = blk.tile([96, L], bf16, tag="dtcat")
            nc.scalar.activation(out=dtcat[0:R], in_=dt_f, func=AF.Copy)
            nc.vector.tensor_sub(dtcat[R : 2 * R], dt_f, dtcat[0:R])
            nc.vector.tensor_copy(dtcat[2 * R : 3 * R], dtcat[0:R])

            dtu = blk.tile([R, L], bf16, tag="dtu")
            uD = blk.tile([R, L], bf16, tag="uD")
            nc.vector.tensor_mul(dtu, dt_f, u_f)
            nc.vector.tensor_scalar_mul(uD, u_f, dvec_s[:, i : i + 1])
            # stage dtu in DRAM: DMA partition-broadcast needs a DRAM source
            nc.sync.dma_start(out=dtu_d[i], in_=dtu)

            B_bc = blk.tile([128, L], bf16, tag="B_bc")
            C_bc = blk.tile([128, L], bf16, tag="C_bc")
            nc.vector.tensor_copy(B_bc[0:64], bc_sb[0:N])
            nc.vector.tensor_copy(B_bc[64:128], bc_sb[0:N])
            nc.vector.tensor_copy(C_bc[0:64], bc_sb[N:128])
            nc.vector.tensor_copy(C_bc[64:128], bc_sb[N:128])

            y_f = blk.tile([R, L], bf16, tag="y_f")
            lastcol = blk.tile([128, NK], bf16, tag="lastcol")

            for h in range(2):
                hsl = slice(HALF * h, HALF * (h + 1))
                psy = [
                    ps_y.tile([R, CHUNK], f32, tag="ps_y", name=f"psy_{i}_{h}_{c4}")
                    for c4 in range(4)
                ]
                for k in range(NK):
                    dtu_bc = work.tile([128, HALF], bf16, tag="dtu_bc")
                    nc.sync.dma_start(
                        out=dtu_bc[0:64],
                        in_=dtu_d[i, 2 * k : 2 * k + 1, hsl].partition_broadcast(64),
                    )
                    nc.sync.dma_start(
                        out=dtu_bc[64:128],
                        in_=dtu_d[i, 2 * k + 1 : 2 * k + 2, hsl].partition_broadcast(64),
                    )
                    dA = work.tile([128, HALF], bf16, tag="dA")
                    for m2 in range(4):
                        msl_d = slice(CHUNK * m2, CHUNK * (m2 + 1))
                        msl_s = slice(HALF * h + CHUNK * m2, HALF * h + CHUNK * (m2 + 1))
                        psm = ps_m.tile([128, CHUNK], f32, tag="ps_m")
                        nc.tensor.matmul(
                            psm, amask_s[:, i, k], dtcat[:, msl_s],
                            start=True, stop=True,
                        )
                        nc.scalar.activation(out=dA[:, msl_d], in_=psm, func=AF.Exp)
                    wk = work.tile([128, HALF], bf16, tag="wk")
                    nc.vector.tensor_mul(wk, dtu_bc, B_bc[:, hsl])
                    Ht = hpool.tile([128, HALF], bf16, tag="H")
                    init = 0.0 if h == 0 else lastcol[:, k : k + 1]
                    nc.vector.tensor_tensor_scan(
                        out=Ht, data0=dA, data1=wk, initial=init,
                        op0=OP.mult, op1=OP.add,
                    )
                    if h == 0:
                        nc.vector.tensor_copy(
                            lastcol[:, k : k + 1], Ht[:, HALF - 1 : HALF]
                        )
                    nc.sync.dma_start(out=states_o[i, k, :, hsl], in_=Ht)
                    # y partial reduction: z = H * C_bc, then selector matmul
                    for c4 in range(4):
                        sl_d = slice(CHUNK * c4, CHUNK * (c4 + 1))
                        sl_f = slice(
                            HALF * h + CHUNK * c4, HALF * h + CHUNK * (c4 + 1)
                        )
                        zk = zpool.tile([128, CHUNK], bf16, tag="z")
                        nc.vector.tensor_mul(zk, Ht[:, sl_d], C_bc[:, sl_f])
                        nc.tensor.matmul(
                            psy[c4], sel32_s[:, k], zk,
                            start=(k == 0), stop=(k == NK - 1),
                        )
                for c4 in range(4):
                    sl_f = slice(HALF * h + CHUNK * c4, HALF * h + CHUNK * (c4 + 1))
                    nc.vector.tensor_add(y_f[:, sl_f], psy[c4], uD[:, sl_f])

            # out-projection + residual
            for c8 in range(NCHUNK):
                sl = slice(CHUNK * c8, CHUNK * (c8 + 1))
                pso = ps_a.tile([E, CHUNK], f32, tag="ps_a")
                nc.tensor.matmul(pso, outwT_s[:, i], y_f[:, sl], start=True, stop=True)
                nc.vector.tensor_add(Xout[:, sl], pso, Xin[:, sl])

        Xf = Xs[NB % 2]
        nc.sync.dma_start(out=p3_o, in_=Xf)

        # --- p4 conv
        tail = ctx.enter_context(tc.tile_pool(name="tail", bufs=1))
        xpad4 = tail.tile([E, 58 * 58], f32, tag="xpad4")
        nc.vector.memset(xpad4, 0.0)
        xpad4_r = _pad_view(xpad4, 58)
        nc.vector.tensor_copy(
            xpad4_r[:, 1:57, 1:57], Xf.rearrange("c (h w) -> c h w", h=H1, w=W1)
        )
        xpad5 = tail.tile([C4, 30 * 30], f32, tag="xpad5")
        nc.vector.memset(xpad5, 0.0)
        xpad5_r = _pad_view(xpad5, 30)
        for c2 in range(2):
            ps = ps_b.tile([C4, CHUNK], f32, tag="ps_b")
            for tap in range(9):
                di, dj = tap // 3, tap % 3
                v = _tap_view(xpad4_r, di, dj, H4)  # [E, 28, 28, 1]
                rhs = v[:, 14 * c2 : 14 * c2 + 14]
                nc.tensor.matmul(
                    ps, p4w_s[:, tap], rhs, start=(tap == 0), stop=(tap == 8)
                )
            yb4 = tail.tile([C4, CHUNK], f32, tag="yb4")
            sg4 = tail.tile([C4, CHUNK], f32, tag="sg4")
            nc.scalar.activation(out=yb4, in_=ps, func=AF.Identity, bias=p4b_s)
            nc.scalar.activation(out=sg4, in_=ps, func=AF.Sigmoid, bias=p4b_s)
            nc.vector.tensor_mul(
                xpad5_r[:, 1 + 14 * c2 : 15 + 14 * c2, 1:29],
                yb4.rearrange("c (i j) -> c i j", i=14, j=W4),
                sg4.rearrange("c (i j) -> c i j", i=14, j=W4),
            )
        p4c = tail.tile([C4, H4 * W4], f32, tag="p4c")
        nc.vector.tensor_copy(
            p4c.rearrange("c (h w) -> c h w", h=H4, w=W4), xpad5_r[:, 1:29, 1:29]
        )
        nc.sync.dma_start(out=p4_o, in_=p4c)

        # --- p5 conv
        p5buf = tail.tile([128, 2, H5 * W5], f32, tag="p5buf")
        for hf in range(2):
            ps = ps_b.tile([128, H5 * W5], f32, tag="ps_b")
            for tap in range(9):
                di, dj = tap // 3, tap % 3
                v = _tap_view(xpad5_r, di, dj, H5)  # [C4, 14, 14, 1]
                nc.tensor.matmul(
                    ps, p5w_s[:, hf, tap], v, start=(tap == 0), stop=(tap == 8)
                )
            yb5 = tail.tile([C4, H5 * W5], f32, tag="yb5")
            sg5 = tail.tile([C4, H5 * W5], f32, tag="sg5")
            nc.scalar.activation(
                out=yb5, in_=ps, func=AF.Identity, bias=p5b_s[:, hf : hf + 1]
            )
            nc.scalar.activation(
                out=sg5, in_=ps, func=AF.Sigmoid, bias=p5b_s[:, hf : hf + 1]
            )
            nc.vector.tensor_mul(p5buf[:, hf], yb5, sg5)
            nc.sync.dma_start(out=p5_o[hf], in_=p5buf[:, hf])

    nc.compile()
    return nc


_CACHE = {}


def _get_program():
    if "nc" not in _CACHE:
        _CACHE["nc"] = build_program()
    return _CACHE["nc"]


def _run(inputs, trace=False, **kw):
    nc = _get_program()
    consts = _prep_consts(inputs)
    x = np.asarray(inputs["x"], np.float32)

    in_maps = []
    for b in range(B):
        m = dict(consts)
        m["x_in"] = _f32(x[b].reshape(CIN, HIN * WIN))
        in_maps.append(m)

    from concourse.bass_utils import run_bass_kernel_spmd

    res = run_bass_kernel_spmd(
        nc, in_maps, core_ids=list(range(B)), trace=trace, **kw
    )
    outs = res.results

    p3 = np.stack([outs[b]["p3_o"].reshape(E, H1, W1) for b in range(B)])
    p4 = np.stack([outs[b]["p4_o"].reshape(C4, H4, W4) for b in range(B)])
    p5 = np.stack(
        [outs[b]["p5_o"].reshape(C5, H5, W5) for b in range(B)]
    )
    states = np.stack(
        [
            outs[b]["states_o"]
            .astype(np.float32)
            .reshape(NB, NK, 2, N, L)
            .transpose(0, 4, 1, 2, 3)
            .reshape(NB, L, R, N)
            for b in range(B)
        ],
        axis=1,
    )
    return (p3, p4, p5, states), res


def kernel(**inputs):
    return _run(inputs)[0]


# revision 49
# speedup vs baseline: 1.1137x; 1.1137x over previous
"""Trainium2 Bass kernel for nn_DLRMambaBackbone.

Strategy: data-parallel over batch (8 batch elems -> 8 NeuronCores). Each core
runs the full backbone for one batch element:
  stem conv(s2) + BN/SiLU -> 3x SS2D selective-scan blocks -> p4/p5 convs.

Per-core layout: activations channel-major [C, L] in SBUF. The selective scan
runs on the vector engine's tensor_tensor_scan over 16 [128, L] tiles whose
partitions are (r, n) lane pairs (2 r's x 64 n's per tile). dA = exp(dt * A)
is built by a K=6 bf16 hi/lo split matmul (exact to ~2^-17) + ACT exp.
The state sequence is DMA'd out rn-major as bf16 and transposed/upcast on the
host (a device-side transpose would need a slow scatter DMA or extra engine
passes).
"""

import os
import sys

import numpy as np

for _p in ("/opt/trn_rl_repo", "/root/.axon_site/_ro/trn_rl_repo"):
    if os.path.isdir(_p) and _p not in sys.path:
        sys.path.insert(0, _p)

import ml_dtypes

BF16 = ml_dtypes.bfloat16

# ---------------- problem geometry (hardcoded) ----------------
B = 8
CIN, HIN, WIN = 32, 112, 112
E = 64            # channels in SS2D trunk
H1 = W1 = 56
L = H1 * W1       # 3136 tokens
R, N, NB = 32, 64, 3
C4, C5 = 128, 256
H4 = W4 = 28
H5 = W5 = 14
NK = 16           # rn tiles: 16 x (2 r's x 64 n's)
CHUNK = 392       # 7 rows of 56 pixels; 3136 = 8*392
NCHUNK = 8
HALF = L // 2     # 1568
EPS = 1e-5


def _f32(x):
    return np.ascontiguousarray(np.asarray(x, dtype=np.float32))


def _bf16(x):
    return np.ascontiguousarray(np.asarray(x, dtype=np.float32).astype(BF16))


def _prep_consts(inp):
    """Host-side weight transforms (all tiny)."""
    c = {}
    # stem: fold BN scale into conv weights, BN shift into bias
    sg = inp["stem_g"] / np.sqrt(inp["stem_v"] + EPS)
    sb = inp["stem_b"] - inp["stem_m"] * sg
    w = inp["stem_w"]  # [E, CIN, 3, 3]
    stemw = np.empty((9, CIN, E), np.float32)
    for di in range(3):
        for dj in range(3):
            stemw[di * 3 + dj] = (w[:, :, di, dj] * sg[:, None]).T
    c["stemw"] = _f32(stemw)
    c["stemb"] = _f32(sb.reshape(E, 1))

    # SS2D blocks
    A = -np.exp(np.asarray(inp["blk_A_log"], np.float64))  # [NB, R, N]
    A = A.astype(np.float32)
    A_hi = A.astype(BF16).astype(np.float32)
    A_lo = (A - A_hi).astype(BF16).astype(np.float32)
    # K=96 masked lhsT per (block, k-tile): rows 0-31 pair with dthi (coef A_hi),
    # rows 32-63 with dtlo (A_hi), rows 64-95 with dthi again (A_lo).
    amask = np.zeros((NB, NK, 96, 128), np.float32)
    for i in range(NB):
        for k in range(NK):
            for j in range(2):
                r = 2 * k + j
                cs = slice(64 * j, 64 * j + 64)
                amask[i, k, r, cs] = A_hi[i, r]
                amask[i, k, 32 + r, cs] = A_hi[i, r]
                amask[i, k, 64 + r, cs] = A_lo[i, r]
    c["amask"] = _bf16(amask.transpose(2, 0, 1, 3))  # -> [96, NB, NK, 128]

    wud = np.empty((E, NB, 64), np.float32)   # cols: [u (R) | dtraw (R)]
    wbc = np.empty((E, NB, 128), np.float32)  # cols: [B (N) | C (N)]
    outwT = np.empty((R, NB, E), np.float32)
    for i in range(NB):
        wud[:, i, :R] = inp["blk_in_w"][i].T
        wud[:, i, R:] = inp["blk_dt_w"][i].T
        wbc[:, i, :N] = inp["blk_B_w"][i].T
        wbc[:, i, N:] = inp["blk_C_w"][i].T
        outwT[:, i, :] = inp["blk_out_w"][i].T
    c["wud"] = _f32(wud)
    c["wbc"] = _f32(wbc)
    c["outwT"] = _bf16(outwT)
    c["dtb"] = _f32(np.asarray(inp["blk_dt_b"]).T.reshape(R, NB))
    c["dvec"] = _f32(np.asarray(inp["blk_D"]).T.reshape(R, NB))

    # per-k selector: column 2k sums partitions 0-63, column 2k+1 sums 64-127
    sel32 = np.zeros((NK, 128, R), np.float32)
    for k in range(NK):
        sel32[k, :64, 2 * k] = 1.0
        sel32[k, 64:, 2 * k + 1] = 1.0
    c["sel32"] = _bf16(sel32.transpose(1, 0, 2))  # -> [128, NK, R]

    # p4 / p5 convs
    sg4 = inp["p4_g"] / np.sqrt(inp["p4_v"] + EPS)
    c["p4b"] = _f32((inp["p4_b"] - inp["p4_m"] * sg4).reshape(C4, 1))
    w4 = inp["p4_w"]  # [C4, E, 3, 3]
    p4w = np.empty((9, E, C4), np.float32)
    for di in range(3):
        for dj in range(3):
            p4w[di * 3 + dj] = (w4[:, :, di, dj] * sg4[:, None]).T
    c["p4w"] = _f32(p4w)

    sg5 = inp["p5_g"] / np.sqrt(inp["p5_v"] + EPS)
    b5 = inp["p5_b"] - inp["p5_m"] * sg5
    w5 = inp["p5_w"]  # [C5, C4, 3, 3]
    p5w = np.empty((2, 9, C4, 128), np.float32)
    for h in range(2):
        for di in range(3):
            for dj in range(3):
                p5w[h, di * 3 + dj] = (
                    w5[h * 128 : (h + 1) * 128, :, di, dj]
                    * sg5[h * 128 : (h + 1) * 128, None]
                ).T
    c["p5w"] = _f32(p5w)
    c["p5b"] = _f32(np.asarray(b5).reshape(2, 128, 1))
    return c


# ---------------- device program ----------------

def _pad_view(t, h):
    """View a [C, h*h] padded tile as tap-sliceable [C, hh, 2, hh, 2]-ish."""
    return t.rearrange("c (h w) -> c h w", h=h, w=h)


def _tap_view(tr, di, dj, hout):
    """Strided tap view [C, hout, hout] of padded image for a 3x3/stride2 conv."""
    hin = 2 * hout
    v = tr[:, di : di + hin, dj : dj + hin]
    v = v.rearrange("c (i a) (j b) -> c i a j b", a=2, b=2)
    return v[:, :, 0:1, :, 0:1]  # [C, hout, 1, hout, 1]


def build_program():
    import concourse.bass as bass
    import concourse.mybir as mybir
    import concourse.tile as tile
    from concourse import bacc

    f32 = mybir.dt.float32
    f32r = mybir.dt.float32r
    bf16 = mybir.dt.bfloat16
    AF = mybir.ActivationFunctionType
    OP = mybir.AluOpType

    nc = bacc.Bacc("TRN2", target_bir_lowering=False, debug=False)

    # --- DRAM tensors
    def din(name, shape, dt=f32):
        return nc.dram_tensor(name, list(shape), dt, kind="ExternalInput").ap()

    def dout(name, shape, dt=f32):
        return nc.dram_tensor(name, list(shape), dt, kind="ExternalOutput").ap()

    x_in = din("x_in", (CIN, HIN * WIN))
    stemw = din("stemw", (9, CIN, E))
    stemb = din("stemb", (E, 1))
    wud = din("wud", (E, NB, 64))
    wbc = din("wbc", (E, NB, 128))
    dtb = din("dtb", (R, NB))
    dvec = din("dvec", (R, NB))
    amask = din("amask", (96, NB, NK, 128), bf16)
    outwT = din("outwT", (R, NB, E), bf16)
    sel32 = din("sel32", (128, NK, R), bf16)
    p4w = din("p4w", (9, E, C4))
    p4b = din("p4b", (C4, 1))
    p5w = din("p5w", (2, 9, C4, 128))
    p5b = din("p5b", (2, C4, 1))

    dtu_d = nc.dram_tensor("dtu_d", [NB, R, L], bf16, kind="Internal").ap()
    states_o = dout("states_o", (NB, NK, 128, L), bf16)
    p3_o = dout("p3_o", (E, L))
    p4_o = dout("p4_o", (C4, H4 * W4))
    p5_o = dout("p5_o", (2, 128, H5 * W5))

    from contextlib import ExitStack

    with tile.TileContext(nc) as tc, ExitStack() as ctx:
        consts = ctx.enter_context(tc.tile_pool(name="consts", bufs=1))
        big = ctx.enter_context(tc.tile_pool(name="big", bufs=1))
        ps_a = ctx.enter_context(tc.tile_pool(name="ps_a", bufs=1, space="PSUM"))
        ps_b = ctx.enter_context(tc.tile_pool(name="ps_b", bufs=1, space="PSUM"))
        ps_m = ctx.enter_context(tc.tile_pool(name="ps_m", bufs=2, space="PSUM"))
        ps_y = ctx.enter_context(tc.tile_pool(name="ps_y", bufs=4, space="PSUM"))

        # --- load constants to SBUF
        def load_const(name, src, shape, dt, rearr=None):
            t = consts.tile(list(shape), dt, tag=name)
            s = src.rearrange(rearr) if rearr else src
            nc.sync.dma_start(out=t, in_=s)
            return t

        stemw_s = load_const("stemw", stemw, (CIN, 9, E), f32, "t c o -> c t o")
        stemb_s = load_const("stemb", stemb, (E, 1), f32)
        wud_s = load_const("wud", wud, (E, NB, 64), f32)
        wbc_s = load_const("wbc", wbc, (E, NB, 128), f32)
        dtb_s = load_const("dtb", dtb, (R, NB), f32)
        dvec_s = load_const("dvec", dvec, (R, NB), f32)
        amask_s = load_const("amask", amask, (96, NB, NK, 128), bf16)
        outwT_s = load_const("outwT", outwT, (R, NB, E), bf16)
        sel32_s = load_const("sel32", sel32, (128, NK, R), bf16)
        p4w_s = load_const("p4w", p4w, (E, 9, C4), f32, "t c o -> c t o")
        p4b_s = load_const("p4b", p4b, (C4, 1), f32)
        p5w_s = load_const("p5w", p5w, (C4, 2, 9, 128), f32, "h t c o -> c h t o")
        p5b_s = load_const("p5b", p5b, (C4, 2), f32, "h c one -> c (h one)")

        # --- stem conv
        X0 = big.tile([E, L], f32, tag="X0")
        X1 = big.tile([E, L], f32, tag="X1")
        Xs = [X0, X1]

        with tc.tile_pool(name="stempad", bufs=1) as stempad:
            xpad = stempad.tile([CIN, 114 * 114], f32, tag="xpad")
            nc.vector.memset(xpad, 0.0)
            xpad_r = _pad_view(xpad, 114)
            xtmp = stempad.tile([CIN, HIN * WIN], f32, tag="xtmp")
            nc.sync.dma_start(out=xtmp, in_=x_in)
            nc.vector.tensor_copy(
                xpad_r[:, 1:113, 1:113],
                xtmp.rearrange("c (h w) -> c h w", h=HIN, w=WIN),
            )
            for c8 in range(NCHUNK):
                ps = ps_a.tile([E, CHUNK], f32, tag="ps_a")
                for tap in range(9):
                    di, dj = tap // 3, tap % 3
                    v = _tap_view(xpad_r, di, dj, H1)  # [CIN, 56, 1, 56, 1]
                    rhs = v[:, 7 * c8 : 7 * c8 + 7]
                    nc.tensor.matmul(
                        ps, stemw_s[:, tap], rhs,
                        start=(tap == 0), stop=(tap == 8),
                    )
                # SiLU = y * sigmoid(y), y = conv + bias
                yb = stempad.tile([E, CHUNK], f32, tag="yb")
                sg = stempad.tile([E, CHUNK], f32, tag="sg")
                nc.scalar.activation(out=yb, in_=ps, func=AF.Identity, bias=stemb_s)
                nc.scalar.activation(out=sg, in_=ps, func=AF.Sigmoid, bias=stemb_s)
                nc.vector.tensor_mul(
                    X0[:, CHUNK * c8 : CHUNK * (c8 + 1)], yb, sg
                )

        # --- SS2D blocks
        blk = ctx.enter_context(tc.tile_pool(name="blk", bufs=1))
        hpool = ctx.enter_context(tc.tile_pool(name="hpool", bufs=4))
        work = ctx.enter_context(tc.tile_pool(name="work", bufs=3))
        zpool = ctx.enter_context(tc.tile_pool(name="zp", bufs=3))
        for i in range(NB):
            Xin = Xs[i % 2]
            Xout = Xs[(i + 1) % 2]

            dt_f = blk.tile([R, L], f32, tag="dt_f")
            et_f = blk.tile([R, L], f32, tag="et_f")
            u_f = blk.tile([R, L], bf16, tag="u_f")
            bc_sb = blk.tile([128, L], bf16, tag="bc_sb")
            for c8 in range(NCHUNK):
                sl = slice(CHUNK * c8, CHUNK * (c8 + 1))
                ps1 = ps_a.tile([64, CHUNK], f32, tag="ps_a")
                nc.tensor.matmul(ps1, wud_s[:, i], Xin[:, sl], start=True, stop=True)
                ps2 = ps_b.tile([128, CHUNK], f32, tag="ps_b")
                nc.tensor.matmul(ps2, wbc_s[:, i], Xin[:, sl], start=True, stop=True)
                # softplus staged as exp (same ACT table set as the big dA
                # exps) + one full-L ln below (2 table switches per block)
                nc.scalar.activation(
                    out=et_f[:, sl], in_=ps1[R:64], func=AF.Exp,
                    bias=dtb_s[:, i : i + 1], scale=1.0,
                )
                nc.scalar.activation(out=u_f[:, sl], in_=ps1[0:R], func=AF.Copy)
                nc.scalar.activation(out=bc_sb[:, sl], in_=ps2, func=AF.Copy)

            nc.scalar.activation(out=dt_f, in_=et_f, func=AF.Ln, bias=1.0)

            # dtcat [96, L]: rows 0-31 dt-hi, 32-63 dt-lo, 64-95 dt-hi (again);
            # pairs with the K=96 masked lhsT amask_s for an exact-ish dt*A.
            dtcat = blk.tile([96, L], bf16, tag="dtcat")
            nc.scalar.activation(out=dtcat[0:R], in_=dt_f, func=AF.Copy)
            nc.vector.tensor_sub(dtcat[R : 2 * R], dt_f, dtcat[0:R])
            nc.vector.tensor_copy(dtcat[2 * R : 3 * R], dtcat[0:R])

            dtu = blk.tile([R, L], bf16, tag="dtu")
            uD = blk.tile([R, L], bf16, tag="uD")
            nc.vector.tensor_mul(dtu, dt_f, u_f)
            nc.vector.tensor_scalar_mul(uD, u_f, dvec_s[:, i : i + 1])
            # stage dtu in DRAM: DMA partition-broadcast needs a DRAM source
            nc.sync.dma_start(out=dtu_d[i], in_=dtu)

            B_bc = blk.tile([128, L], bf16, tag="B_bc")
            C_bc = blk.tile([128, L], bf16, tag="C_bc")
            nc.vector.tensor_copy(B_bc[0:64], bc_sb[0:N])
            nc.vector.tensor_copy(B_bc[64:128], bc_sb[0:N])
            nc.vector.tensor_copy(C_bc[0:64], bc_sb[N:128])
            nc.vector.tensor_copy(C_bc[64:128], bc_sb[N:128])

            y_f = blk.tile([R, L], bf16, tag="y_f")
            lastcol = blk.tile([128, NK], bf16, tag="lastcol")

            for h in range(2):
                hsl = slice(HALF * h, HALF * (h + 1))
                psy = [
                    ps_y.tile([R, CHUNK], f32, tag="ps_y", name=f"psy_{i}_{h}_{c4}")
                    for c4 in range(4)
                ]
                for k in range(NK):
                    dtu_bc = work.tile([128, HALF], bf16, tag="dtu_bc")
                    nc.sync.dma_start(
                        out=dtu_bc[0:64],
                        in_=dtu_d[i, 2 * k : 2 * k + 1, hsl].partition_broadcast(64),
                    )
                    nc.sync.dma_start(
                        out=dtu_bc[64:128],
                        in_=dtu_d[i, 2 * k + 1 : 2 * k + 2, hsl].partition_broadcast(64),
                    )
                    dA = work.tile([128, HALF], bf16, tag="dA")
                    for m2 in range(4):
                        msl_d = slice(CHUNK * m2, CHUNK * (m2 + 1))
                        msl_s = slice(HALF * h + CHUNK * m2, HALF * h + CHUNK * (m2 + 1))
                        psm = ps_m.tile([128, CHUNK], f32, tag="ps_m")
                        nc.tensor.matmul(
                            psm, amask_s[:, i, k], dtcat[:, msl_s],
                            start=True, stop=True,
                        )
                        nc.scalar.activation(out=dA[:, msl_d], in_=psm, func=AF.Exp)
                    wk = work.tile([128, HALF], bf16, tag="wk")
                    nc.vector.tensor_mul(wk, dtu_bc, B_bc[:, hsl])
                    Ht = hpool.tile([128, HALF], bf16, tag="H")
                    init = 0.0 if h == 0 else lastcol[:, k : k + 1]
                    nc.vector.tensor_tensor_scan(
                        out=Ht, data0=dA, data1=wk, initial=init,
                        op0=OP.mult, op1=OP.add,
                    )
                    if h == 0:
                        nc.vector.tensor_copy(
                            lastcol[:, k : k + 1], Ht[:, HALF - 1 : HALF]
                        )
                    nc.sync.dma_start(out=states_o[i, k, :, hsl], in_=Ht)
                    # y partial reduction: z = H * C_bc, then selector matmuls
                    zk = zpool.tile([128, HALF], bf16, tag="z")
                    nc.vector.tensor_mul(zk, Ht, C_bc[:, hsl])
                    for c4 in range(4):
                        nc.tensor.matmul(
                            psy[c4], sel32_s[:, k],
                            zk[:, CHUNK * c4 : CHUNK * (c4 + 1)],
                            start=(k == 0), stop=(k == NK - 1),
                        )
                for c4 in range(4):
                    sl_f = slice(HALF * h + CHUNK * c4, HALF * h + CHUNK * (c4 + 1))
                    nc.vector.tensor_add(y_f[:, sl_f], psy[c4], uD[:, sl_f])

            # out-projection + residual
            for c8 in range(NCHUNK):
                sl = slice(CHUNK * c8, CHUNK * (c8 + 1))
                pso = ps_a.tile([E, CHUNK], f32, tag="ps_a")
                nc.tensor.matmul(pso, outwT_s[:, i], y_f[:, sl], start=True, stop=True)
                nc.vector.tensor_add(Xout[:, sl], pso, Xin[:, sl])

        Xf = Xs[NB % 2]
        nc.sync.dma_start(out=p3_o, in_=Xf)

        # --- p4 conv
        tail = ctx.enter_context(tc.tile_pool(name="tail", bufs=1))
        xpad4 = tail.tile([E, 58 * 58], f32, tag="xpad4")
        nc.vector.memset(xpad4, 0.0)
        xpad4_r = _pad_view(xpad4, 58)
        nc.vector.tensor_copy(
            xpad4_r[:, 1:57, 1:57], Xf.rearrange("c (h w) -> c h w", h=H1, w=W1)
        )
        xpad5 = tail.tile([C4, 30 * 30], f32, tag="xpad5")
        nc.vector.memset(xpad5, 0.0)
        xpad5_r = _pad_view(xpad5, 30)
        for c2 in range(2):
            ps = ps_b.tile([C4, CHUNK], f32, tag="ps_b")
            for tap in range(9):
                di, dj = tap // 3, tap % 3
                v = _tap_view(xpad4_r, di, dj, H4)  # [E, 28, 28, 1]
                rhs = v[:, 14 * c2 : 14 * c2 + 14]
                nc.tensor.matmul(
                    ps, p4w_s[:, tap], rhs,
                    start=(tap == 0), stop=(tap == 8),
                )
            yb4 = tail.tile([C4, CHUNK], f32, tag="yb4")
            sg4 = tail.tile([C4, CHUNK], f32, tag="sg4")
            nc.scalar.activation(out=yb4, in_=ps, func=AF.Identity, bias=p4b_s)
            nc.scalar.activation(out=sg4, in_=ps, func=AF.Sigmoid, bias=p4b_s)
            nc.vector.tensor_mul(
                xpad5_r[:, 1 + 14 * c2 : 15 + 14 * c2, 1:29],
                yb4.rearrange("c (i j) -> c i j", i=14, j=W4),
                sg4.rearrange("c (i j) -> c i j", i=14, j=W4),
            )
        p4c = tail.tile([C4, H4 * W4], f32, tag="p4c")
        nc.vector.tensor_copy(
            p4c.rearrange("c (h w) -> c h w", h=H4, w=W4), xpad5_r[:, 1:29, 1:29]
        )
        nc.sync.dma_start(out=p4_o, in_=p4c)

        # --- p5 conv
        p5buf = tail.tile([128, 2, H5 * W5], f32, tag="p5buf")
        for hf in range(2):
            ps = ps_b.tile([128, H5 * W5], f32, tag="ps_b")
            for tap in range(9):
                di, dj = tap // 3, tap % 3
                v = _tap_view(xpad5_r, di, dj, H5)  # [C4, 14, 14, 1]
                nc.tensor.matmul(
                    ps, p5w_s[:, hf, tap], v,
                    start=(tap == 0), stop=(tap == 8),
                )
            yb5 = tail.tile([C4, H5 * W5], f32, tag="yb5")
            sg5 = tail.tile([C4, H5 * W5], f32, tag="sg5")
            nc.scalar.activation(
                out=yb5, in_=ps, func=AF.Identity, bias=p5b_s[:, hf : hf + 1]
            )
            nc.scalar.activation(
                out=sg5, in_=ps, func=AF.Sigmoid, bias=p5b_s[:, hf : hf + 1]
            )
            nc.vector.tensor_mul(p5buf[:, hf], yb5, sg5)
            nc.sync.dma_start(out=p5_o[hf], in_=p5buf[:, hf])

    nc.compile()
    return nc


_CACHE = {}


def _get_program():
    if "nc" not in _CACHE:
        _CACHE["nc"] = build_program()
    return _CACHE["nc"]


def _run(inputs, trace=False, **kw):
    nc = _get_program()
    consts = _prep_consts(inputs)
    x = np.asarray(inputs["x"], np.float32)

    in_maps = []
    for b in range(B):
        m = dict(consts)
        m["x_in"] = _f32(x[b].reshape(CIN, HIN * WIN))
        in_maps.append(m)

    from concourse.bass_utils import run_bass_kernel_spmd

    res = run_bass_kernel_spmd(
        nc, in_maps, core_ids=list(range(B)), trace=trace, **kw
    )
    outs = res.results

    p3 = np.stack([outs[b]["p3_o"].reshape(E, H1, W1) for b in range(B)])
    p4 = np.stack([outs[b]["p4_o"].reshape(C4, H4, W4) for b in range(B)])
    p5 = np.stack(
        [outs[b]["p5_o"].reshape(C5, H5, W5) for b in range(B)]
    )
    states = np.stack(
        [
            outs[b]["states_o"]
            .astype(np.float32)
            .reshape(NB, NK, 2, N, L)
            .transpose(0, 4, 1, 2, 3)
            .reshape(NB, L, R, N)
            for b in range(B)
        ],
        axis=1,
    )
    return (p3, p4, p5, states), res


def kernel(**inputs):
    return _run(inputs)[0]
